# revision 1
# baseline (speedup 1.0000x reference)
"""DeepseekV4 Mega-MoE experts layer on 8 Trainium2 NeuronCores.

Strategy (expert-parallel, per sharding hint):
  - 16 experts sharded 2-per-core across 8 cores; each core receives its two
    experts' weights (losslessly converted: mxfp4*ue8m0 dequant values are
    exactly representable in TRN fp8_e4m3 for w13 and bf16 for w2).
  - Staging fp8 quantization of hidden_states is replicated on every core
    (direct fp32->fp8e4 cast; bit-identical to the reference group-scaled
    round trip except for deep-subnormal values, rel err ~1e-4).
  - Tokens are gathered per expert on-device with a one-hot matmul (the
    "all-to-all"), expert MLP runs on the gathered subset, and the host sums
    the per-expert outputs (the "combine" all-reduce).

Per-core device pipeline:
  x[512,2048]f32 --ACT cast--> x8 fp8
  x_gT[d,tl] = gather-transpose via PE matmul (lhsT=x8 chunks, rhs=one-hot G)
  h[tl,1536]  = mm1: lhsT=x_gT chunks, rhs=w13T fp8 (accumulate over d)
  a[tl,768]   = silu(h[:, :768]) * h[:, 768:] * comb[tl]   (ACT + DVE)
  aT[i,tl]    = PE transpose
  ye[tl,2048] = mm2: lhsT=aT chunks, rhs=w2T bf16 (accumulate over i)
  DMA ye (bf16) out; host scatter-adds into [512,2048] fp32.
"""

import sys

if "/opt/trn_rl_repo" not in sys.path:
    sys.path.insert(0, "/opt/trn_rl_repo")

import numpy as np
import ml_dtypes

T, D, I, E, TOPK, GROUP = 512, 2048, 768, 16, 8, 32
N_CORES = 8
E_LOC = E // N_CORES  # experts per core

FP8 = ml_dtypes.float8_e4m3      # TRN FP8_EXP4 (max 240) == bass dt.float8e4
BF16 = ml_dtypes.bfloat16

_FP4_TABLE = np.array(
    [0.0, 0.5, 1.0, 1.5, 2.0, 3.0, 4.0, 6.0,
     -0.0, -0.5, -1.0, -1.5, -2.0, -3.0, -4.0, -6.0], dtype=np.float32)


def _dequant_mxfp4(w_packed, sf):
    lo = _FP4_TABLE[w_packed & 0xF]
    hi = _FP4_TABLE[(w_packed >> 4) & 0xF]
    w = np.stack([lo, hi], axis=-1).reshape(*w_packed.shape[:-1], -1)
    s = (sf.astype(np.uint32) << 23).view(np.float32)
    w = w.reshape(*sf.shape, GROUP) * s[..., None]
    return w.reshape(*w_packed.shape[:-1], 2 * w_packed.shape[-1])


_PROGRAM_CACHE = {}


def _build_program(cap, split_waits=True):
    import concourse.bass as bass
    import concourse.mybir as mybir
    import concourse.tile as tile
    from concourse.masks import make_identity

    _TC = tile.TileContext

    def _split_excess_waits(nc):
        # This walrus build accepts only ONE sem-wait per instruction; hoist
        # extra waits onto standalone EventSemaphore (pure-wait) instructions
        # on the same engine, which execute in order ahead of the original.
        n = 0
        for f in nc.m.functions:
            for b in f.blocks:
                out = []
                for ins in b.instructions:
                    si = ins.sync_info
                    waits = list(si.on_wait) if (si and si.on_wait) else []
                    if len(waits) > 1:
                        for k, w in enumerate(waits[:-1]):
                            out.append(mybir.InstEventSemaphore(
                                name=f"{ins.name}-xw{k}", engine=ins.engine,
                                ins=[], outs=[],
                                sync_info=mybir.SyncInfo(
                                    on_wait=[w], on_update=[])))
                            n += 1
                        si.on_wait = waits[-1:]
                    out.append(ins)
                b.instructions = out
        return n

    dt = mybir.dt
    MT = cap // 128            # tl tiles per expert
    DT, FT, IT = D // 128, 2 * I // 512, I // 128   # 16, 3, 6
    TT = T // 128              # 4 token chunks

    nc = bass.Bass()
    x_d = nc.dram_tensor("x", [T, D], dt.float32, kind="ExternalInput")
    g_d = nc.dram_tensor("g", [TT, 128, E_LOC * cap], dt.float8e4, kind="ExternalInput")
    w13_d = nc.dram_tensor("w13t", [E_LOC, DT, 128, 2 * I], dt.float8e4, kind="ExternalInput")
    w2_d = nc.dram_tensor("w2t", [E_LOC, IT, 128, D], dt.float8e4, kind="ExternalInput")
    comb_d = nc.dram_tensor("combg", [E_LOC, MT, 128, 1], dt.float32, kind="ExternalInput")
    ye_d = nc.dram_tensor("ye", [E_LOC, cap, D], dt.bfloat16, kind="ExternalOutput")
    JH = DT // 2   # j tiles per xgT/w13 part (split for DMA/compute pipelining)

    with _TC(nc) as tc:
        with (
            tc.tile_pool(name="const", bufs=1) as constp,
            tc.tile_pool(name="xin", bufs=2) as xinp,
            tc.tile_pool(name="x8", bufs=1) as x8p,
            tc.tile_pool(name="wts", bufs=1) as wtsp,
            tc.tile_pool(name="xg", bufs=1) as xgp,
            tc.tile_pool(name="act", bufs=2) as actp,
            tc.tile_pool(name="yout", bufs=1) as youtp,
            tc.tile_pool(name="ps_big", bufs=2, space="PSUM") as psb,
            tc.tile_pool(name="ps_small", bufs=2, space="PSUM") as pss,
        ):
            ident = constp.tile([128, 128], dt.bfloat16)
            make_identity(nc, ident[:])

            # ---- DMAs in consumption order: x/G/comb, then weights ----
            # stage 0: x -> fp8 (replicated staging quantization)
            x8 = x8p.tile([128, TT, D], dt.float8e4)
            for c in range(TT):
                xin = xinp.tile([128, D], dt.float32)
                nc.sync.dma_start(xin[:], x_d[c * 128:(c + 1) * 128, :])
                if c % 2 == 0:
                    nc.scalar.copy(x8[:, c, :], xin[:])
                else:
                    nc.vector.tensor_copy(x8[:, c, :], xin[:])

            # one-hot gather matrix for BOTH experts side by side (scalar ring,
            # runs in parallel with the x stream on the sync ring)
            gmat = constp.tile([128, TT, E_LOC * cap], dt.float8e4, tag="g")
            nc.scalar.dma_start(gmat[:], g_d.rearrange("c p f -> p c f"))
            combg = []
            for e in range(E_LOC):
                cg = constp.tile([128, MT, 1], dt.float32, tag=f"cg_{e}")
                nc.scalar.dma_start(cg[:], comb_d[e].rearrange("m p f -> p m f"))
                combg.append(cg)
            # weights in strict consumption order, split for pipelining
            w13t, w2t = [], []
            for e in range(E_LOC):
                parts = []
                for p in range(2):
                    wt = wtsp.tile([128, JH, 2 * I], dt.float8e4, tag=f"w13_{e}_{p}")
                    nc.sync.dma_start(
                        wt[:], w13_d[e, p * JH:(p + 1) * JH].rearrange("j p f -> p j f"))
                    parts.append(wt)
                w13t.append(parts)
            for e in range(E_LOC):
                w2 = wtsp.tile([128, IT, D], dt.float8e4, tag=f"w2_{e}")
                nc.sync.dma_start(w2[:], w2_d[e].rearrange("k p f -> p k f"))
                w2t.append(w2)

            # ---- stage 1: gather-transpose x8 -> x_gT (both experts at once) ----
            xgT = []
            for p in range(2):
                xg = xgp.tile([128, JH, E_LOC * cap], dt.float8e4, tag=f"xg_{p}")
                xgT.append(xg)
            for j in range(DT):
                pg = pss.tile([128, E_LOC * cap], dt.float32, tag="sm")
                for v in range(TT // 2):
                    # fp8 DoubleRow over two token chunks at once
                    nc.tensor.matmul(
                        pg[:],
                        x8[:, 2 * v:2 * v + 2, j * 128:(j + 1) * 128],
                        gmat[:, 2 * v:2 * v + 2, :],
                        start=(v == 0), stop=(v == TT // 2 - 1),
                        perf_mode=mybir.MatmulPerfMode.DoubleRow)
                nc.scalar.copy(xgT[j // JH][:, j % JH, :], pg[:])

            # ---- stages 2-4: expert MLP front half ----
            aT = []
            for e in range(E_LOC):
                at = actp.tile([128, IT, cap], dt.bfloat16, tag=f"aT_{e}")
                aT.append(at)
            for e in range(E_LOC):
                hs = [psb.tile([128, 2 * I], dt.float32, tag="acc", name=f"h_{e}_{mm}")
                      for mm in range(MT)]
                for u in range(DT // 2):
                    p, uu = (2 * u) // JH, (2 * u) % JH
                    for m in range(MT):
                        for fb in range(FT):
                            # fp8 DoubleRow: contract 256 rows (2 d-chunks) per op
                            nc.tensor.matmul(
                                hs[m][:, fb * 512:(fb + 1) * 512],
                                xgT[p][:, uu:uu + 2,
                                       e * cap + m * 128:e * cap + (m + 1) * 128],
                                w13t[e][p][:, uu:uu + 2, fb * 512:(fb + 1) * 512],
                                start=(u == 0), stop=(u == DT // 2 - 1),
                                perf_mode=mybir.MatmulPerfMode.DoubleRow)
                for m in range(MT):
                    h = hs[m]
                    s = actp.tile([128, I], dt.float32, tag="silu")
                    nc.scalar.activation(
                        s[:], h[:, 0:I], mybir.ActivationFunctionType.Sigmoid)
                    t = actp.tile([128, I], dt.float32, tag="sg")
                    nc.vector.tensor_tensor(
                        t[:], s[:], h[:, 0:I], op=mybir.AluOpType.mult)
                    a = actp.tile([128, I], dt.bfloat16, tag="a")
                    # a = (silu(gate) * comb) * up
                    nc.vector.scalar_tensor_tensor(
                        a[:], t[:], combg[e][:, m, :], h[:, I:2 * I],
                        op0=mybir.AluOpType.mult, op1=mybir.AluOpType.mult)
                    for k in range(IT):
                        pt = pss.tile([128, 128], dt.bfloat16, tag="sm")
                        nc.tensor.transpose(
                            pt[:], a[:, k * 128:(k + 1) * 128], ident[:])
                        nc.vector.tensor_copy(
                            aT[e][:, k, m * 128:(m + 1) * 128], pt[:])

            for e in range(E_LOC):
                ye = youtp.tile([128, MT, D], dt.bfloat16, tag=f"ye_{e}")
                for m in range(MT):
                    for dq in range(4):
                        yh = pss.tile([128, 512], dt.float32, tag="sm")
                        for k in range(IT):
                            nc.tensor.matmul(
                                yh[:],
                                aT[e][:, k, m * 128:(m + 1) * 128],
                                w2t[e][:, k, dq * 512:(dq + 1) * 512],
                                start=(k == 0), stop=(k == IT - 1))
                        nc.vector.tensor_copy(
                            ye[:, m, dq * 512:(dq + 1) * 512], yh[:])
                    nc.scalar.dma_start(
                        ye_d[e].rearrange("(m p) f -> p m f", p=128)[:, m, :],
                        ye[:, m, :])

    nc.finalize()
    if split_waits:
        _split_excess_waits(nc)
    return nc


def kernel(hidden_states, topk_weights, topk_ids, w13_weight, w13_weight_scale,
           w2_weight, w2_weight_scale):
    from concourse.bass_utils import run_bass_kernel_spmd

    x = np.ascontiguousarray(hidden_states, dtype=np.float32)
    tw = np.asarray(topk_weights, dtype=np.float32)
    ti = np.asarray(topk_ids)

    # host routing: combine weights + per-expert token lists
    comb = np.zeros((T, E), np.float32)
    for k in range(TOPK):
        np.add.at(comb, (np.arange(T), ti[:, k]), tw[:, k])
    routed = comb > 0.0
    idx = [np.nonzero(routed[:, e])[0] for e in range(E)]
    counts = [len(ix) for ix in idx]
    cap = max(128, -(-max(counts) // 128) * 128)

    if cap not in _PROGRAM_CACHE:
        _PROGRAM_CACHE[cap] = _build_program(cap)
    nc = _PROGRAM_CACHE[cap]

    # weights: lossless host conversion (see module docstring)
    w13 = _dequant_mxfp4(np.asarray(w13_weight), np.asarray(w13_weight_scale))
    w2 = _dequant_mxfp4(np.asarray(w2_weight), np.asarray(w2_weight_scale))
    DT, IT, TT, MT = D // 128, I // 128, T // 128, cap // 128

    in_maps = []
    for core in range(N_CORES):
        m = {"x": x}
        g = np.zeros((T, E_LOC * cap), FP8)
        cg = np.zeros((E_LOC, cap), np.float32)
        w13t = np.zeros((E_LOC, DT, 128, 2 * I), FP8)
        w2t = np.zeros((E_LOC, IT, 128, D), FP8)
        for le in range(E_LOC):
            e = core * E_LOC + le
            ix = idx[e]
            g[ix, le * cap + np.arange(len(ix))] = FP8(1.0)
            cg[le, :len(ix)] = comb[ix, e]
            w13t[le] = w13[e].T.astype(FP8).reshape(DT, 128, 2 * I)
            w2t[le] = w2[e].T.astype(FP8).reshape(IT, 128, D)
        m["g"] = np.ascontiguousarray(g.reshape(TT, 128, E_LOC * cap))
        m["combg"] = np.ascontiguousarray(cg.reshape(E_LOC, MT, 128, 1))
        m["w13t"] = w13t
        m["w2t"] = w2t
        in_maps.append(m)

    res = run_bass_kernel_spmd(nc, in_maps, list(range(N_CORES)))

    out = np.zeros((T, D), np.float32)
    for core in range(N_CORES):
        ye = np.asarray(res.results[core]["ye"], dtype=np.float32)
        for le in range(E_LOC):
            e = core * E_LOC + le
            ix = idx[e]
            out[ix] += ye[le, :len(ix)]
    return out



# revision 9
# speedup vs baseline: 1.1977x; 1.1977x over previous
"""DeepseekV4 Mega-MoE experts layer on 8 Trainium2 NeuronCores.

Strategy (expert-parallel, per sharding hint):
  - 16 experts sharded 2-per-core across 8 cores; each core receives its two
    experts' weights (losslessly converted: mxfp4*ue8m0 dequant values are
    exactly representable in TRN fp8_e4m3 for both w13 and w2).
  - Staging fp8 quantization of hidden_states is replicated on every core
    (direct fp32->fp8e4 cast; matches the reference group-scaled round trip
    except for deep-subnormal values, rel err ~6e-4).
  - Tokens are gathered per expert on-device with a one-hot matmul (the
    "all-to-all"), expert MLP runs on the gathered subset, and the host sums
    the per-expert outputs (the "combine" all-reduce).

Device schedule (v2 — overlap-optimized):
  x streams in 4 column blocks of 512 features; each block is quantized to
  fp8 and immediately gather-transposed, so mm1 starts after ~1/4 of x has
  arrived instead of all of it.  All input DMAs are issued in consumption
  order on one ring so weights arrive just-in-time behind the x blocks.
  mm1 runs m-serial per expert so each h PSUM accumulator is released
  (by the act chain) before the next expert needs its slot — no PE bubble
  at expert boundaries.  PE queue order:
    [gather-b0, mm1-e0-m0-u01, gather-b1, mm1-e0-m0-u23, ..., mm1-e0-m1,
     mm1-e1-m0, mm1-e1-m1, trans-e0-m0, mm2-e0-m0, ..., trans-e1-m1,
     mm2-e1-m1]
  Acts (sigmoid/mult/scale-mult) are sliced in halves and pipelined across
  ACT/Pool/DVE.  mm2 outputs are copied and DMA'd out per 512-column piece.
"""

import sys

if "/opt/trn_rl_repo" not in sys.path:
    sys.path.insert(0, "/opt/trn_rl_repo")

import numpy as np
import ml_dtypes

T, D, I, E, TOPK, GROUP = 512, 2048, 768, 16, 8, 32
N_CORES = 8
E_LOC = E // N_CORES  # experts per core

FP8 = ml_dtypes.float8_e4m3      # TRN FP8_EXP4 (max 240) == bass dt.float8e4
BF16 = ml_dtypes.bfloat16

_FP4_TABLE = np.array(
    [0.0, 0.5, 1.0, 1.5, 2.0, 3.0, 4.0, 6.0,
     -0.0, -0.5, -1.0, -1.5, -2.0, -3.0, -4.0, -6.0], dtype=np.float32)


def _dequant_mxfp4(w_packed, sf):
    lo = _FP4_TABLE[w_packed & 0xF]
    hi = _FP4_TABLE[(w_packed >> 4) & 0xF]
    w = np.stack([lo, hi], axis=-1).reshape(*w_packed.shape[:-1], -1)
    s = (sf.astype(np.uint32) << 23).view(np.float32)
    w = w.reshape(*sf.shape, GROUP) * s[..., None]
    return w.reshape(*w_packed.shape[:-1], 2 * w_packed.shape[-1])


_PROGRAM_CACHE = {}


def _build_program(cap, split_waits=True):
    import concourse.bass as bass
    import concourse.mybir as mybir
    import concourse.tile as tile
    from concourse.masks import make_identity

    _TC = tile.TileContext

    def _split_excess_waits(nc):
        # This walrus build accepts only ONE sem-wait per instruction; hoist
        # extra waits onto standalone EventSemaphore (pure-wait) instructions
        # on the same engine, which execute in order ahead of the original.
        n = 0
        for f in nc.m.functions:
            for b in f.blocks:
                out = []
                for ins in b.instructions:
                    si = ins.sync_info
                    waits = list(si.on_wait) if (si and si.on_wait) else []
                    if len(waits) > 1:
                        for k, w in enumerate(waits[:-1]):
                            out.append(mybir.InstEventSemaphore(
                                name=f"{ins.name}-xw{k}", engine=ins.engine,
                                ins=[], outs=[],
                                sync_info=mybir.SyncInfo(
                                    on_wait=[w], on_update=[])))
                            n += 1
                        si.on_wait = waits[-1:]
                    out.append(ins)
                b.instructions = out
        return n

    dt = mybir.dt
    MT = cap // 128                 # token tiles per expert
    SLOTS = E_LOC * cap             # gathered slots across local experts
    DT, FT, IT = D // 128, 2 * I // 512, I // 128   # 16, 3, 6
    TT = T // 128                   # 4 token chunks
    NB = 4                          # x column blocks
    BD = D // NB                    # feature cols per block (512)
    BJ = DT // NB                   # d-chunks (j) per block (4)
    NS = 4                          # w13 DMA slabs per expert
    SJ = DT // NS                   # j per w13 slab (4)
    DQ = D // 512                   # mm2 output column pieces (4)
    HI = I // 2                     # act half-slice width (384)

    nc = bass.Bass()
    x_d = nc.dram_tensor("x", [T, D], dt.float32, kind="ExternalInput")
    g_d = nc.dram_tensor("g", [TT, 128, SLOTS], dt.float8e4, kind="ExternalInput")
    w13_d = nc.dram_tensor("w13t", [E_LOC, DT, 128, 2 * I], dt.float8e4, kind="ExternalInput")
    w2_d = nc.dram_tensor("w2t", [E_LOC, IT, 128, D], dt.float8e4, kind="ExternalInput")
    comb_d = nc.dram_tensor("combg", [128, E_LOC * MT], dt.float32, kind="ExternalInput")
    ye_d = nc.dram_tensor("ye", [E_LOC, cap, D], dt.bfloat16, kind="ExternalOutput")

    with _TC(nc) as tc:
        with (
            tc.tile_pool(name="const", bufs=1) as constp,
            tc.tile_pool(name="xin", bufs=3) as xinp,
            tc.tile_pool(name="x8", bufs=2) as x8p,
            tc.tile_pool(name="xg", bufs=1) as xgp,
            tc.tile_pool(name="wts", bufs=1) as wtsp,
            tc.tile_pool(name="act", bufs=2) as actp,
            tc.tile_pool(name="apool", bufs=4) as apool,
            tc.tile_pool(name="yout", bufs=4) as youtp,
            tc.tile_pool(name="ps_g", bufs=2, space="PSUM") as psg,
            tc.tile_pool(name="ps_h", bufs=2, space="PSUM") as psh,
        ):
            # ---- DMA pushes, consumption order ----
            # gather matrix + combine weights on the scalar ring (parallel
            # with the sync ring's x/w13 pushes below)
            gmat = constp.tile([128, TT, SLOTS], dt.float8e4, tag="g")
            nc.scalar.dma_start(gmat[:], g_d.rearrange("c p f -> p c f"))
            combg = constp.tile([128, E_LOC * MT], dt.float32, tag="cg")
            nc.scalar.dma_start(combg[:], comb_d[:])

            # sync ring: x blocks interleaved with w13-e0 slabs, then
            # w13-e1 slabs, then w2 per (expert, output-column piece).
            xr = x_d.rearrange("(c p) (b f) -> b p c f", p=128, f=BD)
            xin = []
            w13t = [[None] * NS for _ in range(E_LOC)]
            for b in range(NB):
                xi = xinp.tile([128, TT, BD], dt.float32, tag="xin",
                               name=f"xin_{b}")
                nc.sync.dma_start(xi[:], xr[b])
                xin.append(xi)
                wt = wtsp.tile([128, SJ, 2 * I], dt.float8e4, tag=f"w13_0_{b}")
                nc.sync.dma_start(
                    wt[:], w13_d[0, b * SJ:(b + 1) * SJ].rearrange("j p f -> p j f"))
                w13t[0][b] = wt
            for e in range(1, E_LOC):
                for s in range(NS):
                    wt = wtsp.tile([128, SJ, 2 * I], dt.float8e4, tag=f"w13_{e}_{s}")
                    nc.sync.dma_start(
                        wt[:], w13_d[e, s * SJ:(s + 1) * SJ].rearrange("j p f -> p j f"))
                    w13t[e][s] = wt
            w2t = [[None] * DQ for _ in range(E_LOC)]
            for e in range(E_LOC):
                for dq in range(DQ):
                    wt = wtsp.tile([128, IT, 512], dt.float8e4, tag=f"w2_{e}_{dq}")
                    nc.sync.dma_start(
                        wt[:],
                        w2_d[e, :, :, dq * 512:(dq + 1) * 512].rearrange("k p f -> p k f"))
                    w2t[e][dq] = wt

            ident = constp.tile([128, 128], dt.bfloat16)
            make_identity(nc, ident[:])

            # ---- phase 1: stream x blocks -> quantize -> gather-transpose,
            # with mm1-e0-m0 interleaved per block pair of d-chunks ----
            xgT = xgp.tile([128, DT, SLOTS], dt.float8e4, tag="xgT")
            h = {}

            def mm1_ops(e, m, u):
                # one DoubleRow contraction step (256 d) for all FT f-blocks
                hm = h[(e, m)]
                s, jj = divmod(2 * u, SJ)
                for fb in range(FT):
                    nc.tensor.matmul(
                        hm[:, fb * 512:(fb + 1) * 512],
                        xgT[:, 2 * u:2 * u + 2,
                            e * cap + m * 128:e * cap + (m + 1) * 128],
                        w13t[e][s][:, jj:jj + 2, fb * 512:(fb + 1) * 512],
                        start=(u == 0), stop=(u == DT // 2 - 1),
                        perf_mode=mybir.MatmulPerfMode.DoubleRow)

            h[(0, 0)] = psh.tile([128, 2 * I], dt.float32, tag="acc", name="h_0_0")
            qeng = [nc.scalar, nc.vector]
            for b in range(NB):
                x8 = x8p.tile([128, TT, BD], dt.float8e4, tag="x8",
                              name=f"x8_{b}")
                for c in range(TT):
                    eng = qeng[(b * TT + c) % 2]
                    if eng is nc.scalar:
                        eng.copy(x8[:, c, :], xin[b][:, c, :])
                    else:
                        eng.tensor_copy(x8[:, c, :], xin[b][:, c, :])
                for jj in range(BJ):
                    j = b * BJ + jj
                    pg = psg.tile([128, SLOTS], dt.float32, tag="sm", name=f"pg_{j}")
                    for v in range(TT // 2):
                        nc.tensor.matmul(
                            pg[:],
                            x8[:, 2 * v:2 * v + 2, jj * 128:(jj + 1) * 128],
                            gmat[:, 2 * v:2 * v + 2, :],
                            start=(v == 0), stop=(v == TT // 2 - 1),
                            perf_mode=mybir.MatmulPerfMode.DoubleRow)
                    if j % 2 == 0:
                        nc.vector.tensor_copy(xgT[:, j, :], pg[:])
                    else:
                        nc.scalar.copy(xgT[:, j, :], pg[:])
                # mm1 e0 m0 consumes the two u-steps this block enabled
                for u in (2 * b, 2 * b + 1):
                    mm1_ops(0, 0, u)

            # ---- phase 2: remaining mm1 (m-serial per expert) + acts ----
            a_tiles = {}

            def acts(e, m):
                # a = (silu(gate) * comb) * up, in half-slices pipelined
                # across ACT (silu) and DVE (scale-mult)
                hm = h[(e, m)]
                cg = combg[:, e * MT + m:e * MT + m + 1]
                a = apool.tile([128, I], dt.bfloat16, tag="a", name=f"a_{e}_{m}")
                for hf in range(2):
                    sl = slice(hf * HI, (hf + 1) * HI)
                    sg = actp.tile([128, HI], dt.float32, tag=f"sg_{hf}",
                                   name=f"sg_{e}_{m}_{hf}")
                    nc.scalar.activation(
                        sg[:], hm[:, hf * HI:(hf + 1) * HI],
                        mybir.ActivationFunctionType.Silu)
                    nc.vector.scalar_tensor_tensor(
                        a[:, sl], sg[:], cg, hm[:, I + hf * HI:I + (hf + 1) * HI],
                        op0=mybir.AluOpType.mult, op1=mybir.AluOpType.mult)
                a_tiles[(e, m)] = a

            first = True
            for e in range(E_LOC):
                for m in range(MT):
                    if not first:
                        h[(e, m)] = psh.tile([128, 2 * I], dt.float32, tag="acc",
                                             name=f"h_{e}_{m}")
                        for u in range(DT // 2):
                            mm1_ops(e, m, u)
                    first = False
                    acts(e, m)

            # ---- phase 3: transpose a + mm2 + output, per (e, m) ----
            ceng = [nc.vector, nc.scalar]
            for e in range(E_LOC):
                for m in range(MT):
                    a = a_tiles[(e, m)]
                    aT = actp.tile([128, IT, 128], dt.bfloat16, tag="aT",
                                   name=f"aT_{e}_{m}")
                    for k in range(IT):
                        pt = psg.tile([128, 128], dt.bfloat16, tag="sm",
                                      name=f"pt_{e}_{m}_{k}")
                        nc.tensor.transpose(
                            pt[:], a[:, k * 128:(k + 1) * 128], ident[:])
                        eng = ceng[k % 2]
                        eng.tensor_copy(aT[:, k, :], pt[:]) if eng is nc.vector \
                            else eng.copy(aT[:, k, :], pt[:])
                    for dq in range(DQ):
                        yh = psg.tile([128, 512], dt.float32, tag="sm",
                                      name=f"yh_{e}_{m}_{dq}")
                        for k in range(IT):
                            nc.tensor.matmul(
                                yh[:],
                                aT[:, k, :],
                                w2t[e][dq][:, k, :],
                                start=(k == 0), stop=(k == IT - 1))
                        ye = youtp.tile([128, 512], dt.bfloat16, tag="ye",
                                        name=f"ye_{e}_{m}_{dq}")
                        eng = ceng[dq % 2]
                        eng.tensor_copy(ye[:], yh[:]) if eng is nc.vector \
                            else eng.copy(ye[:], yh[:])
                        nc.gpsimd.dma_start(
                            ye_d[e].rearrange("(m p) f -> p m f", p=128)
                            [:, m, dq * 512:(dq + 1) * 512],
                            ye[:])

    nc.finalize()
    if split_waits:
        _split_excess_waits(nc)
    return nc


def kernel(hidden_states, topk_weights, topk_ids, w13_weight, w13_weight_scale,
           w2_weight, w2_weight_scale):
    from concourse.bass_utils import run_bass_kernel_spmd

    x = np.ascontiguousarray(hidden_states, dtype=np.float32)
    tw = np.asarray(topk_weights, dtype=np.float32)
    ti = np.asarray(topk_ids)

    # host routing: combine weights + per-expert token lists
    comb = np.zeros((T, E), np.float32)
    for k in range(TOPK):
        np.add.at(comb, (np.arange(T), ti[:, k]), tw[:, k])
    routed = comb > 0.0
    idx = [np.nonzero(routed[:, e])[0] for e in range(E)]
    counts = [len(ix) for ix in idx]
    cap = max(128, -(-max(counts) // 128) * 128)

    if cap not in _PROGRAM_CACHE:
        _PROGRAM_CACHE[cap] = _build_program(cap)
    nc = _PROGRAM_CACHE[cap]

    # weights: lossless host conversion (see module docstring)
    w13 = _dequant_mxfp4(np.asarray(w13_weight), np.asarray(w13_weight_scale))
    w2 = _dequant_mxfp4(np.asarray(w2_weight), np.asarray(w2_weight_scale))
    DT, IT, TT, MT = D // 128, I // 128, T // 128, cap // 128

    in_maps = []
    for core in range(N_CORES):
        m = {"x": x}
        g = np.zeros((T, E_LOC * cap), FP8)
        cg = np.zeros((128, E_LOC * MT), np.float32)
        w13t = np.zeros((E_LOC, DT, 128, 2 * I), FP8)
        w2t = np.zeros((E_LOC, IT, 128, D), FP8)
        for le in range(E_LOC):
            e = core * E_LOC + le
            ix = idx[e]
            g[ix, le * cap + np.arange(len(ix))] = FP8(1.0)
            cvals = np.zeros(cap, np.float32)
            cvals[:len(ix)] = comb[ix, e]
            cg[:, le * MT:(le + 1) * MT] = cvals.reshape(MT, 128).T
            w13t[le] = w13[e].T.astype(FP8).reshape(DT, 128, 2 * I)
            w2t[le] = w2[e].T.astype(FP8).reshape(IT, 128, D)
        m["g"] = np.ascontiguousarray(g.reshape(TT, 128, E_LOC * cap))
        m["combg"] = np.ascontiguousarray(cg)
        m["w13t"] = w13t
        m["w2t"] = w2t
        in_maps.append(m)

    res = run_bass_kernel_spmd(nc, in_maps, list(range(N_CORES)))

    out = np.zeros((T, D), np.float32)
    for core in range(N_CORES):
        ye = np.asarray(res.results[core]["ye"], dtype=np.float32)
        for le in range(E_LOC):
            e = core * E_LOC + le
            ix = idx[e]
            out[ix] += ye[le, :len(ix)]
    return out
